# revision 41
# baseline (speedup 1.0000x reference)
"""Trainium2 Bass kernel for AdjacencyAttention.

Computation (per batch element b of B=4096):
    lhs = (x[b] @ (W1*W2))            # (64,)
    rhs = (x[b] @ W3)                 # (64,)
    t   = tanh(outer(lhs, rhs) + bs)  # (64,64)
    s   = Vs @ t                      # (64,64)
    out = softmax(s.flatten()).reshape(64,64)

Sharding: pure data parallelism over the batch axis across 8 cores.

Per-core dataflow (512 batches, 32 per iteration, 16 iterations):
  - DMA a (128, 16, 256) x row-tile block (32 batches).
  - PE-transpose the 32 (128,128) chunks into PSUM (4 groups of 8),
    cast back to SBUF as fp32r (xT: d on partitions) on DVE.
  - Dot products on the PE: 4 chains x 2 fp32r matmuls (K=256) of
    [w2p w3] against xT -> lhs/rhs for all 2048 rows.
  - Stage dot PSUM to SBUF (ACT), then scatter: lhs rows into W1/W2
    (rect DMAs), rhs onto 16 partitions (rect DMA) then the offset-0
    diagonal DMA into RIe/RIo.
  - Outer product + bias as a 2-matmul fp32r K-accumulation per
    512-column half: W1(80,128) rows 0..16 = even-batch lhs, rows
    16..80 = bs^T; RIe rows 0..16 = even rhs block-diagonal, rows
    16..80 = identity64 tiled; W2/RIo (16,...) the same for odd
    batches.  Output (128, 2, 512) = (parity, i) x (pair, k).
  - ACT tanh -> 2 fp32r matmuls with block-diag Vs^T -> ACT exp.
  - Softmax: DVE segmented sum (128,16); PE ones-block-diag matmul
    sums over partitions per parity AND broadcasts in one op; DVE
    reciprocal; pool broadcast-AP multiply.
  - Softmax is computed without max subtraction: |s| <= ~64 << 88 (fp32
    exp overflow), checked against the reference in test.py.
"""

import sys

import numpy as np

for _p in ("/opt/trn_rl_repo",):
    if _p not in sys.path:
        sys.path.insert(0, _p)

B, N, D = 4096, 64, 256
NCORES = 8
BS = B // NCORES  # 512 batches per core
G2 = 32           # batches per main-loop iteration
NG = BS // G2     # 16 iterations
TPB = G2 * N // 128  # x row-tiles (128 rows) per iteration = 16

_CACHE = {}


def build_bass(repeat=1):
    import concourse.bacc as bacc
    import concourse.bass as bass
    import concourse.mybir as mybir
    import concourse.tile as tile
    from concourse.masks import make_identity

    fp32 = mybir.dt.float32
    f32r = mybir.dt.float32r
    mult = mybir.AluOpType.mult
    add = mybir.AluOpType.add

    nc = bacc.Bacc(
        "TRN2",
        target_bir_lowering=False,
        debug=False,
        enable_asserts=True,
        num_devices=NCORES,
    )
    x_d = nc.dram_tensor("x", (BS * N, D), fp32, kind="ExternalInput").ap()
    w2_d = nc.dram_tensor("w2p", (1, D), fp32, kind="ExternalInput").ap()
    w3_d = nc.dram_tensor("w3", (1, D), fp32, kind="ExternalInput").ap()
    bs_d = nc.dram_tensor("bsm", (N, N), fp32, kind="ExternalInput").ap()
    vs_d = nc.dram_tensor("vs", (N, N), fp32, kind="ExternalInput").ap()
    out_d = nc.dram_tensor("out", (BS, N, N), fp32, kind="ExternalOutput").ap()

    with tile.TileContext(nc) as tc:
        with (
            tc.tile_pool(name="consts", bufs=1) as consts,
            tc.tile_pool(name="xin", bufs=2) as xin_p,
            tc.tile_pool(name="xT", bufs=2) as xT_p,
            tc.tile_pool(name="stg", bufs=3) as stg_p,
            tc.tile_pool(name="rst", bufs=3) as rst_p,
            tc.tile_pool(name="tsb", bufs=2) as tsb_p,
            tc.tile_pool(name="esb", bufs=3) as esb_p,
            tc.tile_pool(name="small", bufs=3) as small_p,
            tc.tile_pool(name="asb", bufs=2) as asb_p,
            tc.tile_pool(name="psT", bufs=2, space="PSUM") as psT_p,
            tc.tile_pool(name="psD", bufs=2, space="PSUM") as psD_p,
            tc.tile_pool(name="big", bufs=1, space="PSUM") as big_p,
        ):
            # prefetch the first x blocks before constant setup so the big
            # loads overlap the init
            prefetched = {}
            for g0 in range(2):
                xt0 = xin_p.tile([128, TPB, D], fp32)
                src0 = x_d[g0 * G2 * N : (g0 + 1) * G2 * N, :].rearrange(
                    "(t p) d -> p t d", p=128
                )
                nc.scalar.dma_start(xt0[:], src0)
                prefetched[g0] = xt0

            # ---- constants ----
            ident = consts.tile([128, 128], fp32)
            make_identity(nc, ident[:])

            # [w2p w3] with d on partitions: (128, chunk, which)
            w23f = consts.tile([128, 2, 2], fp32)
            for c in range(2):
                nc.sync.dma_start(w23f[:, c, 0:1], w2_d[:, 128 * c : 128 * (c + 1)])
                nc.sync.dma_start(w23f[:, c, 1:2], w3_d[:, 128 * c : 128 * (c + 1)])
            w23b = consts.tile([128, 2, 2], f32r)
            nc.vector.tensor_copy(w23b[:], w23f[:])

            bs_sb = consts.tile([N, N], fp32)
            nc.sync.dma_start(bs_sb[:], bs_d)
            vs_sb = consts.tile([N, N], fp32)
            nc.sync.dma_start(vs_sb[:], vs_d)

            # transposed constants via PE transpose (rounded to fp32r)
            bsT = consts.tile([N, N], f32r)
            vsT = consts.tile([N, N], f32r)
            for src, dstt in ((bs_sb, bsT), (vs_sb, vsT)):
                ptt = big_p.tile([128, 2, 512], fp32, tag="big")
                nc.tensor.transpose(ptt[:N, 0, :N], src[:], ident[:N, :N])
                nc.vector.tensor_copy(dstt[:], ptt[:N, 0, :N])

            zeros = consts.tile([128, 1024], fp32)
            nc.vector.memset(zeros[:], 0.0)

            # block-diag Vs^T (128, 128): out[(h,i)] = sum_j VsT[j,i] t[(h,j)]
            VsBD = consts.tile([128, 128], f32r)
            nc.vector.tensor_copy(VsBD[:], zeros[:, 0:128])
            nc.sync.dma_start(VsBD[0:64, 0:64], vsT[:])
            nc.sync.dma_start(VsBD[64:128, 64:128], vsT[:])

            # ones block-diag for per-parity partition sum + broadcast
            ones_bd = consts.tile([128, 128], fp32)
            nc.vector.memset(ones_bd[:], 0.0)
            nc.vector.memset(ones_bd[0:64, 0:64], 1.0)
            nc.vector.memset(ones_bd[64:128, 64:128], 1.0)

            # identity64 tiled 16x along free dim (for bs part of RIe)
            iwide = consts.tile([N, TPB, N], f32r)
            for r in range(TPB):
                nc.vector.tensor_copy(iwide[:, r, :], ident[:N, :N])

            # ---- persistent W / RI operand rings ----
            # Outer product + bias as a 2-matmul K-accumulation:
            #   W1 (80, 128): rows 0..16 = lhs of even batches (cols 0..64),
            #     rows 16..80 = bs^T in both column halves;
            #   RIe (80, 1024): rows 0..16 = even-batch rhs block-diagonal
            #     (row t nonzero at cols 64t..64t+64), rows 16..80 =
            #     identity64 tiled 16x;
            #   W2/RIo (16, ...): the same for odd batches.
            # pt = W1^T @ RIe + W2^T @ RIo holds outer(lhs,rhs)+bs for all
            # 32 batches: (parity, i) x (pair, k).
            W1_ring, W2_ring, RIe_ring, RIo_ring = [], [], [], []
            for ri in range(3):
                W1t = consts.tile([80, 128], f32r, name=f"W1{ri}", tag=f"W1{ri}")
                nc.vector.tensor_copy(W1t[:], zeros[0:80, 0:128])
                nc.sync.dma_start(W1t[16:80, 0:64], bsT[:])
                nc.sync.dma_start(W1t[16:80, 64:128], bsT[:])
                W2t = consts.tile([16, 128], f32r, name=f"W2{ri}", tag=f"W2{ri}")
                nc.vector.tensor_copy(W2t[:], zeros[0:16, 0:128])
                RIet = consts.tile([80, 1024], f32r, name=f"RIe{ri}",
                                   tag=f"RIe{ri}")
                nc.vector.tensor_copy(RIet[:], zeros[0:80, :])
                nc.sync.dma_start(
                    RIet[16:80, :], iwide[:].rearrange("p a b -> p (a b)")
                )
                RIot = consts.tile([16, 1024], f32r, name=f"RIo{ri}",
                                   tag=f"RIo{ri}")
                nc.vector.tensor_copy(RIot[:], zeros[0:16, :])
                W1_ring.append(W1t)
                W2_ring.append(W2t)
                RIe_ring.append(RIet)
                RIo_ring.append(RIot)

            def diag_ap(RIt):
                # dims (t16, i64) -> RI[t, t*64 + i]  (diagonal, offset 0)
                return bass.AP(RIt[:].tensor, 0, [[1088, 16], [1, 64]])

            # ---- main loop: software pipeline ----
            st = {}

            def s0_load(g):
                if g in prefetched:
                    st[g] = {"xt": prefetched.pop(g)}
                    return
                xt = xin_p.tile([128, TPB, D], fp32)
                src = x_d[(g % NG) * G2 * N : ((g % NG) + 1) * G2 * N, :].rearrange(
                    "(t p) d -> p t d", p=128
                )
                nc.scalar.dma_start(xt[:], src)
                st[g] = {"xt": xt}

            def s1_xT(g):
                # 32 PE transposes of (128,128) chunks -> PSUM, 8 per 2-bank
                # group; cast back to xTb fp32r (d' on partitions).
                xt = st[g].pop("xt")
                xTb = xT_p.tile([128, TPB, 2, 128], f32r)
                for q in range(4):
                    psT = psT_p.tile([128, 8, 128], fp32, tag="psT")
                    for j in range(8):
                        t = 4 * q + j // 2
                        c = j % 2
                        nc.tensor.matmul(
                            psT[:, j, :],
                            lhsT=xt[:, t, 128 * c : 128 * (c + 1)],
                            rhs=ident[:],
                            is_transpose=True,
                            skip_group_check=True,
                        )
                    dst = xTb[:, 4 * q : 4 * q + 4, :, :].rearrange(
                        "p t c r -> p (t c) r"
                    )
                    nc.vector.tensor_copy(dst, psT[:])
                st[g]["xTb"] = xTb

            def s2_dots(g):
                # lhs/rhs dots for 2048 rows: 4 chains x 2 matmuls (K=256
                # over 2 chunks), N=512 each, fp32r.
                xTb = st[g].pop("xTb")
                psDs = []
                for nh in range(4):
                    psD = psD_p.tile([2, 512], fp32, tag="psD")
                    for c in range(2):
                        nc.tensor.matmul(
                            psD[:],
                            lhsT=w23b[:, c, :],
                            rhs=xTb[:, 4 * nh : 4 * nh + 4, c, :],
                            start=(c == 0),
                            stop=(c == 1),
                        )
                    psDs.append(psD)
                # stage layout: [lr(part), nh4, t4, half, i]
                stage = stg_p.tile([2, 4, 4, 2, 64], f32r)
                for nh in range(4):
                    nc.scalar.copy(
                        stage[:, nh, :, :, :],
                        psDs[nh][:].rearrange("p (t h i) -> p t h i", t=4, h=2),
                    )
                # hop 1: scatter the rhs dot row onto 16 partitions (t-major)
                rstage = rst_p.tile([TPB, 2, 64], f32r)
                nc.sync.dma_start(
                    rstage[:],
                    stage[1:2].rearrange("p n t h i -> p (n t) (h i)"),
                )
                st[g]["stage"] = stage
                st[g]["rstage"] = rstage

            def s2b_wri(g):
                stage = st[g].pop("stage")
                rstage = st[g].pop("rstage")
                ring = g % 3
                nc.sync.dma_start(
                    W1_ring[ring][0:16, 0:64], stage[0:1, :, :, 0, :]
                )
                nc.sync.dma_start(
                    W2_ring[ring][0:16, 64:128], stage[0:1, :, :, 1, :]
                )
                nc.gpsimd.dma_start(diag_ap(RIe_ring[ring]), rstage[:, 0, :])
                nc.gpsimd.dma_start(diag_ap(RIo_ring[ring]), rstage[:, 1, :])
                st[g]["ring"] = ring

            def s3_outer(g):
                ring = st[g].pop("ring")
                pt = big_p.tile([128, 2, 512], fp32, tag="big")
                for u in range(2):
                    nc.tensor.matmul(
                        pt[:, u, :],
                        lhsT=W1_ring[ring][:],
                        rhs=RIe_ring[ring][:, 512 * u : 512 * (u + 1)],
                        start=True,
                        stop=False,
                    )
                    nc.tensor.matmul(
                        pt[:, u, :],
                        lhsT=W2_ring[ring][:],
                        rhs=RIo_ring[ring][:, 512 * u : 512 * (u + 1)],
                        start=False,
                        stop=True,
                    )
                t_sb = tsb_p.tile([128, 1024], f32r)
                nc.scalar.activation(
                    t_sb[:],
                    pt[:].rearrange("p a b -> p (a b)"),
                    mybir.ActivationFunctionType.Tanh,
                )
                st[g]["t_sb"] = t_sb

            def s4_vs(g):
                t_sb = st[g].pop("t_sb")
                ps = big_p.tile([128, 2, 512], fp32, tag="big")
                for u in range(2):
                    nc.tensor.matmul(
                        ps[:, u, :],
                        lhsT=VsBD[:],
                        rhs=t_sb[:, 512 * u : 512 * (u + 1)],
                        start=True,
                        stop=True,
                    )
                e_sb = esb_p.tile([128, TPB, 64], fp32)
                nc.scalar.activation(
                    e_sb[:].rearrange("p a b -> p (a b)"),
                    ps[:].rearrange("p a b -> p (a b)"),
                    mybir.ActivationFunctionType.Exp,
                )
                st[g]["e_sb"] = e_sb

            def s5a_sum(g):
                e_sb = st[g]["e_sb"]
                sums = small_p.tile([128, TPB], fp32, tag="sums")
                nc.vector.tensor_reduce(
                    out=sums[:], in_=e_sb[:], axis=mybir.AxisListType.X, op=add
                )
                st[g]["sums"] = sums

            def s5_soft(g):
                e_sb = st[g].pop("e_sb")
                sums = st[g].pop("sums")
                sb_ps = psD_p.tile([128, TPB], fp32, tag="psD")
                nc.tensor.matmul(
                    sb_ps[:], lhsT=ones_bd[:], rhs=sums[:], start=True, stop=True
                )
                rb = small_p.tile([128, TPB], fp32, tag="rb")
                nc.vector.reciprocal(rb[:], sb_ps[:])
                a2 = asb_p.tile([128, TPB, 64], fp32, tag="a2", name="a2")
                rb_b = rb[:].unsqueeze(-1).broadcast_to((128, TPB, 64))
                nc.gpsimd.tensor_tensor(out=a2[:], in0=e_sb[:], in1=rb_b, op=mult)
                # batch index within a group of 32 is 2*b + h; partition is
                # (h, i)
                dst = out_d[(g % NG) * G2 : (g % NG + 1) * G2].rearrange(
                    "(b two) i k -> two i b k", two=2
                )
                nc.sync.dma_start(dst[0], a2[0:64, :, :])
                nc.scalar.dma_start(dst[1], a2[64:128, :, :])
                del st[g]

            lags = [(0, s0_load), (1, s1_xT), (2, s2_dots), (3, s2b_wri),
                    (4, s3_outer), (5, s4_vs), (6, s5a_sum), (7, s5_soft)]
            total = NG * repeat
            for i in range(total + 7):
                for lag, fn in lags:
                    g = i - lag
                    if 0 <= g < total:
                        fn(g)

    nc.compile()
    return nc


def _get_nc():
    if "nc" not in _CACHE:
        _CACHE["nc"] = build_bass()
    return _CACHE["nc"]


def make_in_maps(x, W1, W2, W3, bs, Vs):
    x = np.ascontiguousarray(np.asarray(x, dtype=np.float32))
    w2p = (np.asarray(W2, np.float32)[:, 0] * np.asarray(W1, np.float32)[0]).reshape(1, D)
    w3 = np.asarray(W3, np.float32).reshape(1, D)
    bsm = np.ascontiguousarray(np.asarray(bs, np.float32).reshape(N, N))
    vs = np.ascontiguousarray(np.asarray(Vs, np.float32))
    in_maps = []
    for c in range(NCORES):
        shard = x[c * BS : (c + 1) * BS].reshape(BS * N, D)
        in_maps.append(
            {
                "x": np.ascontiguousarray(shard),
                "w2p": w2p,
                "w3": w3,
                "bsm": bsm,
                "vs": vs,
            }
        )
    return in_maps


def kernel(x, W1, W2, W3, bs, Vs):
    from concourse.bass_utils import run_bass_kernel_spmd

    nc = _get_nc()
    in_maps = make_in_maps(x, W1, W2, W3, bs, Vs)
    res = run_bass_kernel_spmd(nc, in_maps, core_ids=list(range(NCORES)))
    out = np.concatenate([r["out"] for r in res.results], axis=0)
    return out.astype(np.float32)


if __name__ == "__main__":
    nc = build_bass()
    print("built ok:", nc)


# revision 42
# speedup vs baseline: 1.1081x; 1.1081x over previous
"""Trainium2 Bass kernel for AdjacencyAttention.

Computation (per batch element b of B=4096):
    lhs = (x[b] @ (W1*W2))            # (64,)
    rhs = (x[b] @ W3)                 # (64,)
    t   = tanh(outer(lhs, rhs) + bs)  # (64,64)
    s   = Vs @ t                      # (64,64)
    out = softmax(s.flatten()).reshape(64,64)

Sharding: pure data parallelism over the batch axis across 8 cores.

Per-core dataflow (512 batches, 32 per iteration, 16 iterations):
  - DMA a (128, 16, 256) x row-tile block (32 batches).
  - PE-transpose the 32 (128,128) chunks into PSUM (4 groups of 8),
    cast back to SBUF as fp32r (xT: d on partitions) on DVE.
  - Dot products on the PE: 4 chains x 2 fp32r matmuls (K=256) of
    [w2p w3] against xT -> lhs/rhs for all 2048 rows.
  - Stage dot PSUM to SBUF (ACT), then scatter: lhs rows into W1/W2
    (rect DMAs), rhs onto 16 partitions (rect DMA) then the offset-0
    diagonal DMA into RIe/RIo.
  - Outer product + bias as a 2-matmul fp32r K-accumulation per
    512-column half: W1(80,128) rows 0..16 = even-batch lhs, rows
    16..80 = bs^T; RIe rows 0..16 = even rhs block-diagonal, rows
    16..80 = identity64 tiled; W2/RIo (16,...) the same for odd
    batches.  Output (128, 2, 512) = (parity, i) x (pair, k).
  - ACT tanh -> 2 fp32r matmuls with block-diag Vs^T -> ACT exp.
  - Softmax: DVE segmented sum (128,16); PE ones-block-diag matmul
    sums over partitions per parity AND broadcasts in one op; DVE
    reciprocal; pool broadcast-AP multiply.
  - Softmax is computed without max subtraction: |s| <= ~64 << 88 (fp32
    exp overflow), checked against the reference in test.py.
"""

import sys

import numpy as np

for _p in ("/opt/trn_rl_repo",):
    if _p not in sys.path:
        sys.path.insert(0, _p)

B, N, D = 4096, 64, 256
NCORES = 8
BS = B // NCORES  # 512 batches per core
G2 = 32           # batches per main-loop iteration
NG = BS // G2     # 16 iterations
TPB = G2 * N // 128  # x row-tiles (128 rows) per iteration = 16

_CACHE = {}


def build_bass(repeat=1):
    import concourse.bacc as bacc
    import concourse.bass as bass
    import concourse.mybir as mybir
    import concourse.tile as tile
    from concourse.masks import make_identity

    fp32 = mybir.dt.float32
    f32r = mybir.dt.float32r
    mult = mybir.AluOpType.mult
    add = mybir.AluOpType.add

    nc = bacc.Bacc(
        "TRN2",
        target_bir_lowering=False,
        debug=False,
        enable_asserts=True,
        num_devices=NCORES,
    )
    x_d = nc.dram_tensor("x", (BS * N, D), fp32, kind="ExternalInput").ap()
    w2_d = nc.dram_tensor("w2p", (1, D), fp32, kind="ExternalInput").ap()
    w3_d = nc.dram_tensor("w3", (1, D), fp32, kind="ExternalInput").ap()
    bs_d = nc.dram_tensor("bsm", (N, N), fp32, kind="ExternalInput").ap()
    vs_d = nc.dram_tensor("vs", (N, N), fp32, kind="ExternalInput").ap()
    out_d = nc.dram_tensor("out", (BS, N, N), fp32, kind="ExternalOutput").ap()

    with tile.TileContext(nc) as tc:
        with (
            tc.tile_pool(name="consts", bufs=1) as consts,
            tc.tile_pool(name="xin", bufs=2) as xin_p,
            tc.tile_pool(name="xT", bufs=2) as xT_p,
            tc.tile_pool(name="stg", bufs=3) as stg_p,
            tc.tile_pool(name="rst", bufs=3) as rst_p,
            tc.tile_pool(name="tsb", bufs=2) as tsb_p,
            tc.tile_pool(name="esb", bufs=3) as esb_p,
            tc.tile_pool(name="small", bufs=3) as small_p,
            tc.tile_pool(name="asb", bufs=2) as asb_p,
            tc.tile_pool(name="psT", bufs=2, space="PSUM") as psT_p,
            tc.tile_pool(name="psD", bufs=2, space="PSUM") as psD_p,
            tc.tile_pool(name="big", bufs=1, space="PSUM") as big_p,
        ):
            # prefetch the first x blocks before constant setup so the big
            # loads overlap the init
            prefetched = {}
            for g0 in range(2):
                xt0 = xin_p.tile([128, TPB, D], fp32)
                src0 = x_d[g0 * G2 * N : (g0 + 1) * G2 * N, :].rearrange(
                    "(t p) d -> p t d", p=128
                )
                nc.scalar.dma_start(xt0[:], src0)
                prefetched[g0] = xt0

            # ---- constants ----
            ident = consts.tile([128, 128], fp32)
            make_identity(nc, ident[:])

            # [w2p w3] with d on partitions: (128, chunk, which)
            w23f = consts.tile([128, 2, 2], fp32)
            for c in range(2):
                nc.sync.dma_start(w23f[:, c, 0:1], w2_d[:, 128 * c : 128 * (c + 1)])
                nc.sync.dma_start(w23f[:, c, 1:2], w3_d[:, 128 * c : 128 * (c + 1)])
            w23b = consts.tile([128, 2, 2], f32r)
            nc.vector.tensor_copy(w23b[:], w23f[:])

            bs_sb = consts.tile([N, N], fp32)
            nc.sync.dma_start(bs_sb[:], bs_d)
            vs_sb = consts.tile([N, N], fp32)
            nc.sync.dma_start(vs_sb[:], vs_d)

            # transposed constants via PE transpose (rounded to fp32r)
            bsT = consts.tile([N, N], f32r)
            vsT = consts.tile([N, N], f32r)
            for src, dstt in ((bs_sb, bsT), (vs_sb, vsT)):
                ptt = big_p.tile([128, 2, 512], fp32, tag="big")
                nc.tensor.transpose(ptt[:N, 0, :N], src[:], ident[:N, :N])
                nc.vector.tensor_copy(dstt[:], ptt[:N, 0, :N])

            zeros = consts.tile([128, 1024], fp32)
            nc.vector.memset(zeros[:], 0.0)

            # block-diag Vs^T (128, 128): out[(h,i)] = sum_j VsT[j,i] t[(h,j)]
            VsBD = consts.tile([128, 128], f32r)
            nc.vector.tensor_copy(VsBD[:], zeros[:, 0:128])
            nc.sync.dma_start(VsBD[0:64, 0:64], vsT[:])
            nc.sync.dma_start(VsBD[64:128, 64:128], vsT[:])

            # ones block-diag for per-parity partition sum + broadcast
            ones_bd = consts.tile([128, 128], fp32)
            nc.vector.memset(ones_bd[:], 0.0)
            nc.vector.memset(ones_bd[0:64, 0:64], 1.0)
            nc.vector.memset(ones_bd[64:128, 64:128], 1.0)

            # identity64 tiled 16x along free dim (for bs part of RIe)
            iwide = consts.tile([N, TPB, N], f32r)
            for r in range(TPB):
                nc.vector.tensor_copy(iwide[:, r, :], ident[:N, :N])

            # ---- persistent W / RI operand rings ----
            # Outer product + bias as a 2-matmul K-accumulation:
            #   W1 (80, 128): rows 0..16 = lhs of even batches (cols 0..64),
            #     rows 16..80 = bs^T in both column halves;
            #   RIe (80, 1024): rows 0..16 = even-batch rhs block-diagonal
            #     (row t nonzero at cols 64t..64t+64), rows 16..80 =
            #     identity64 tiled 16x;
            #   W2/RIo (16, ...): the same for odd batches.
            # pt = W1^T @ RIe + W2^T @ RIo holds outer(lhs,rhs)+bs for all
            # 32 batches: (parity, i) x (pair, k).
            W1_ring, W2_ring, RIe_ring, RIo_ring = [], [], [], []
            for ri in range(3):
                W1t = consts.tile([80, 128], f32r, name=f"W1{ri}", tag=f"W1{ri}")
                nc.vector.tensor_copy(W1t[:], zeros[0:80, 0:128])
                nc.sync.dma_start(W1t[16:80, 0:64], bsT[:])
                nc.sync.dma_start(W1t[16:80, 64:128], bsT[:])
                W2t = consts.tile([16, 128], f32r, name=f"W2{ri}", tag=f"W2{ri}")
                nc.vector.tensor_copy(W2t[:], zeros[0:16, 0:128])
                RIet = consts.tile([80, 1024], f32r, name=f"RIe{ri}",
                                   tag=f"RIe{ri}")
                nc.vector.tensor_copy(RIet[:], zeros[0:80, :])
                nc.sync.dma_start(
                    RIet[16:80, :], iwide[:].rearrange("p a b -> p (a b)")
                )
                RIot = consts.tile([16, 1024], f32r, name=f"RIo{ri}",
                                   tag=f"RIo{ri}")
                nc.vector.tensor_copy(RIot[:], zeros[0:16, :])
                W1_ring.append(W1t)
                W2_ring.append(W2t)
                RIe_ring.append(RIet)
                RIo_ring.append(RIot)

            def diag_ap(RIt):
                # dims (t16, i64) -> RI[t, t*64 + i]  (diagonal, offset 0)
                return bass.AP(RIt[:].tensor, 0, [[1088, 16], [1, 64]])

            # ---- main loop: software pipeline ----
            st = {}

            def s0_load(g):
                if g in prefetched:
                    st[g] = {"xt": prefetched.pop(g)}
                    return
                xt = xin_p.tile([128, TPB, D], fp32)
                src = x_d[(g % NG) * G2 * N : ((g % NG) + 1) * G2 * N, :].rearrange(
                    "(t p) d -> p t d", p=128
                )
                nc.scalar.dma_start(xt[:], src)
                st[g] = {"xt": xt}

            def s1_xT(g):
                # 32 PE transposes of (128,128) chunks -> PSUM, 8 per 2-bank
                # group; cast back to xTb fp32r (d' on partitions).
                xt = st[g].pop("xt")
                xTb = xT_p.tile([128, TPB, 2, 128], f32r)
                for q in range(4):
                    psT = psT_p.tile([128, 8, 128], fp32, tag="psT")
                    for j in range(8):
                        t = 4 * q + j // 2
                        c = j % 2
                        nc.tensor.matmul(
                            psT[:, j, :],
                            lhsT=xt[:, t, 128 * c : 128 * (c + 1)],
                            rhs=ident[:],
                            is_transpose=True,
                            skip_group_check=True,
                        )
                    dst = xTb[:, 4 * q : 4 * q + 4, :, :].rearrange(
                        "p t c r -> p (t c) r"
                    )
                    nc.vector.tensor_copy(dst, psT[:])
                st[g]["xTb"] = xTb

            def s2_dots(g):
                # lhs/rhs dots for 2048 rows: 4 chains x 2 matmuls (K=256
                # over 2 chunks), N=512 each, fp32r.
                xTb = st[g].pop("xTb")
                psDs = []
                for nh in range(4):
                    psD = psD_p.tile([2, 512], fp32, tag="psD")
                    for c in range(2):
                        nc.tensor.matmul(
                            psD[:],
                            lhsT=w23b[:, c, :],
                            rhs=xTb[:, 4 * nh : 4 * nh + 4, c, :],
                            start=(c == 0),
                            stop=(c == 1),
                        )
                    psDs.append(psD)
                # stage layout: [lr(part), nh4, t4, half, i]
                stage = stg_p.tile([2, 4, 4, 2, 64], f32r)
                for nh in range(4):
                    nc.scalar.copy(
                        stage[:, nh, :, :, :],
                        psDs[nh][:].rearrange("p (t h i) -> p t h i", t=4, h=2),
                    )
                # hop 1: scatter the rhs dot row onto 16 partitions (t-major)
                rstage = rst_p.tile([TPB, 2, 64], f32r)
                nc.sync.dma_start(
                    rstage[:],
                    stage[1:2].rearrange("p n t h i -> p (n t) (h i)"),
                )
                st[g]["stage"] = stage
                st[g]["rstage"] = rstage

            def s2b_wri(g):
                stage = st[g].pop("stage")
                rstage = st[g].pop("rstage")
                ring = g % 3
                nc.sync.dma_start(
                    W1_ring[ring][0:16, 0:64], stage[0:1, :, :, 0, :]
                )
                nc.sync.dma_start(
                    W2_ring[ring][0:16, 64:128], stage[0:1, :, :, 1, :]
                )
                nc.gpsimd.dma_start(diag_ap(RIe_ring[ring]), rstage[:, 0, :])
                nc.gpsimd.dma_start(diag_ap(RIo_ring[ring]), rstage[:, 1, :])
                st[g]["ring"] = ring

            def s3_outer(g):
                ring = st[g].pop("ring")
                pt = big_p.tile([128, 2, 512], fp32, tag="big")
                for u in range(2):
                    nc.tensor.matmul(
                        pt[:, u, :],
                        lhsT=W1_ring[ring][:],
                        rhs=RIe_ring[ring][:, 512 * u : 512 * (u + 1)],
                        start=True,
                        stop=False,
                    )
                    nc.tensor.matmul(
                        pt[:, u, :],
                        lhsT=W2_ring[ring][:],
                        rhs=RIo_ring[ring][:, 512 * u : 512 * (u + 1)],
                        start=False,
                        stop=True,
                    )
                t_sb = tsb_p.tile([128, 1024], f32r)
                nc.scalar.activation(
                    t_sb[:],
                    pt[:].rearrange("p a b -> p (a b)"),
                    mybir.ActivationFunctionType.Tanh,
                )
                st[g]["t_sb"] = t_sb

            def s4_vs(g):
                t_sb = st[g].pop("t_sb")
                ps = big_p.tile([128, 2, 512], fp32, tag="big")
                for u in range(2):
                    nc.tensor.matmul(
                        ps[:, u, :],
                        lhsT=VsBD[:],
                        rhs=t_sb[:, 512 * u : 512 * (u + 1)],
                        start=True,
                        stop=True,
                    )
                e_sb = esb_p.tile([128, TPB, 64], fp32)
                nc.scalar.activation(
                    e_sb[:].rearrange("p a b -> p (a b)"),
                    ps[:].rearrange("p a b -> p (a b)"),
                    mybir.ActivationFunctionType.Exp,
                )
                st[g]["e_sb"] = e_sb

            def s5a_sum(g):
                e_sb = st[g]["e_sb"]
                sums = small_p.tile([128, TPB], fp32, tag="sums")
                nc.vector.tensor_reduce(
                    out=sums[:], in_=e_sb[:], axis=mybir.AxisListType.X, op=add
                )
                st[g]["sums"] = sums

            def s5_soft(g):
                e_sb = st[g].pop("e_sb")
                sums = st[g].pop("sums")
                sb_ps = psD_p.tile([128, TPB], fp32, tag="psD")
                nc.tensor.matmul(
                    sb_ps[:], lhsT=ones_bd[:], rhs=sums[:], start=True, stop=True
                )
                rb = small_p.tile([128, TPB], fp32, tag="rb")
                nc.vector.reciprocal(rb[:], sb_ps[:])
                a2 = asb_p.tile([128, TPB, 64], fp32, tag="a2", name="a2")
                rb_b = rb[:].unsqueeze(-1).broadcast_to((128, TPB, 64))
                nc.gpsimd.tensor_tensor(out=a2[:], in0=e_sb[:], in1=rb_b, op=mult)
                # batch index within a group of 32 is 2*b + h; partition is
                # (h, i)
                dst = out_d[(g % NG) * G2 : (g % NG + 1) * G2].rearrange(
                    "(b two) i k -> two i b k", two=2
                )
                for h in range(2):
                    nc.sync.dma_start(dst[h], a2[64 * h : 64 * h + 64, :, :])
                del st[g]

            lags = [(0, s0_load), (1, s1_xT), (2, s2_dots), (3, s2b_wri),
                    (4, s3_outer), (5, s4_vs), (6, s5a_sum), (7, s5_soft)]
            total = NG * repeat
            for i in range(total + 7):
                for lag, fn in lags:
                    g = i - lag
                    if 0 <= g < total:
                        fn(g)

    nc.compile()
    return nc


def _get_nc():
    if "nc" not in _CACHE:
        _CACHE["nc"] = build_bass()
    return _CACHE["nc"]


def make_in_maps(x, W1, W2, W3, bs, Vs):
    x = np.ascontiguousarray(np.asarray(x, dtype=np.float32))
    w2p = (np.asarray(W2, np.float32)[:, 0] * np.asarray(W1, np.float32)[0]).reshape(1, D)
    w3 = np.asarray(W3, np.float32).reshape(1, D)
    bsm = np.ascontiguousarray(np.asarray(bs, np.float32).reshape(N, N))
    vs = np.ascontiguousarray(np.asarray(Vs, np.float32))
    in_maps = []
    for c in range(NCORES):
        shard = x[c * BS : (c + 1) * BS].reshape(BS * N, D)
        in_maps.append(
            {
                "x": np.ascontiguousarray(shard),
                "w2p": w2p,
                "w3": w3,
                "bsm": bsm,
                "vs": vs,
            }
        )
    return in_maps


def kernel(x, W1, W2, W3, bs, Vs):
    from concourse.bass_utils import run_bass_kernel_spmd

    nc = _get_nc()
    in_maps = make_in_maps(x, W1, W2, W3, bs, Vs)
    res = run_bass_kernel_spmd(nc, in_maps, core_ids=list(range(NCORES)))
    out = np.concatenate([r["out"] for r in res.results], axis=0)
    return out.astype(np.float32)


if __name__ == "__main__":
    nc = build_bass()
    print("built ok:", nc)


# revision 44
# speedup vs baseline: 1.2439x; 1.1225x over previous
"""Trainium2 Bass kernel for AdjacencyAttention.

Computation (per batch element b of B=4096):
    lhs = (x[b] @ (W1*W2))            # (64,)
    rhs = (x[b] @ W3)                 # (64,)
    t   = tanh(outer(lhs, rhs) + bs)  # (64,64)
    s   = Vs @ t                      # (64,64)
    out = softmax(s.flatten()).reshape(64,64)

Sharding: pure data parallelism over the batch axis across 8 cores.

Per-core dataflow (512 batches, 32 per iteration, 16 iterations):
  - DMA a (128, 16, 256) x row-tile block (32 batches).
  - PE-transpose the 32 (128,128) chunks into PSUM (4 groups of 8),
    cast back to SBUF as fp32r (xT: d on partitions) on DVE.
  - Dot products on the PE: 4 chains x 2 fp32r matmuls (K=256) of
    [w2p w3] against xT -> lhs/rhs for all 2048 rows.
  - Stage dot PSUM to SBUF (ACT), then scatter: lhs rows into W1/W2
    (rect DMAs), rhs onto 16 partitions (rect DMA) then the offset-0
    diagonal DMA into RIe/RIo.
  - Outer product + bias as a 2-matmul fp32r K-accumulation per
    512-column half: W1(80,128) rows 0..16 = even-batch lhs, rows
    16..80 = bs^T; RIe rows 0..16 = even rhs block-diagonal, rows
    16..80 = identity64 tiled; W2/RIo (16,...) the same for odd
    batches.  Output (128, 2, 512) = (parity, i) x (pair, k).
  - ACT tanh -> 2 fp32r matmuls with block-diag Vs^T -> ACT exp.
  - Softmax: DVE segmented sum (128,16); PE ones-block-diag matmul
    sums over partitions per parity AND broadcasts in one op; DVE
    reciprocal; pool broadcast-AP multiply.
  - Softmax is computed without max subtraction: |s| <= ~64 << 88 (fp32
    exp overflow), checked against the reference in test.py.
"""

import sys

import numpy as np

for _p in ("/opt/trn_rl_repo",):
    if _p not in sys.path:
        sys.path.insert(0, _p)

B, N, D = 4096, 64, 256
NCORES = 8
BS = B // NCORES  # 512 batches per core
G2 = 32           # batches per main-loop iteration
NG = BS // G2     # 16 iterations
TPB = G2 * N // 128  # x row-tiles (128 rows) per iteration = 16

_CACHE = {}


def build_bass(repeat=1):
    import concourse.bacc as bacc
    import concourse.bass as bass
    import concourse.mybir as mybir
    import concourse.tile as tile
    from concourse.masks import make_identity

    fp32 = mybir.dt.float32
    f32r = mybir.dt.float32r
    mult = mybir.AluOpType.mult
    add = mybir.AluOpType.add

    nc = bacc.Bacc(
        "TRN2",
        target_bir_lowering=False,
        debug=False,
        enable_asserts=True,
        num_devices=NCORES,
    )
    x_d = nc.dram_tensor("x", (BS * N, D), fp32, kind="ExternalInput").ap()
    w2_d = nc.dram_tensor("w2p", (1, D), fp32, kind="ExternalInput").ap()
    w3_d = nc.dram_tensor("w3", (1, D), fp32, kind="ExternalInput").ap()
    bs_d = nc.dram_tensor("bsm", (N, N), fp32, kind="ExternalInput").ap()
    vs_d = nc.dram_tensor("vs", (N, N), fp32, kind="ExternalInput").ap()
    out_d = nc.dram_tensor("out", (BS, N, N), fp32, kind="ExternalOutput").ap()

    with tile.TileContext(nc) as tc:
        with (
            tc.tile_pool(name="consts", bufs=1) as consts,
            tc.tile_pool(name="xin", bufs=2) as xin_p,
            tc.tile_pool(name="xT", bufs=2) as xT_p,
            tc.tile_pool(name="stg", bufs=3) as stg_p,
            tc.tile_pool(name="rst", bufs=3) as rst_p,
            tc.tile_pool(name="tsb", bufs=2) as tsb_p,
            tc.tile_pool(name="esb", bufs=3) as esb_p,
            tc.tile_pool(name="small", bufs=3) as small_p,
            tc.tile_pool(name="asb", bufs=2) as asb_p,
            tc.tile_pool(name="psT", bufs=2, space="PSUM") as psT_p,
            tc.tile_pool(name="psD", bufs=2, space="PSUM") as psD_p,
            tc.tile_pool(name="big", bufs=1, space="PSUM") as big_p,
        ):
            # prefetch the first x blocks before constant setup so the big
            # loads overlap the init
            prefetched = {}
            for g0 in range(2):
                xt0 = xin_p.tile([128, TPB, D], fp32)
                src0 = x_d[g0 * G2 * N : (g0 + 1) * G2 * N, :].rearrange(
                    "(t p) d -> p t d", p=128
                )
                nc.scalar.dma_start(xt0[:], src0)
                prefetched[g0] = xt0

            # ---- constants ----
            ident = consts.tile([128, 128], fp32)
            make_identity(nc, ident[:])

            # [w2p w3] with d on partitions: (128, chunk, which)
            w23f = consts.tile([128, 2, 2], fp32)
            for c in range(2):
                nc.sync.dma_start(w23f[:, c, 0:1], w2_d[:, 128 * c : 128 * (c + 1)])
                nc.sync.dma_start(w23f[:, c, 1:2], w3_d[:, 128 * c : 128 * (c + 1)])
            w23b = consts.tile([128, 2, 2], f32r)
            nc.vector.tensor_copy(w23b[:], w23f[:])

            bs_sb = consts.tile([N, N], fp32)
            nc.sync.dma_start(bs_sb[:], bs_d)
            vs_sb = consts.tile([N, N], fp32)
            nc.sync.dma_start(vs_sb[:], vs_d)

            # transposed constants via PE transpose (rounded to fp32r)
            bsT = consts.tile([N, N], f32r)
            vsT = consts.tile([N, N], f32r)
            for src, dstt in ((bs_sb, bsT), (vs_sb, vsT)):
                ptt = big_p.tile([128, 2, 512], fp32, tag="big")
                nc.tensor.transpose(ptt[:N, 0, :N], src[:], ident[:N, :N])
                nc.vector.tensor_copy(dstt[:], ptt[:N, 0, :N])

            zeros = consts.tile([128, 1024], fp32)
            nc.vector.memset(zeros[:], 0.0)

            # block-diag Vs^T (128, 128): out[(h,i)] = sum_j VsT[j,i] t[(h,j)]
            VsBD = consts.tile([128, 128], f32r)
            nc.vector.tensor_copy(VsBD[:], zeros[:, 0:128])
            nc.sync.dma_start(VsBD[0:64, 0:64], vsT[:])
            nc.sync.dma_start(VsBD[64:128, 64:128], vsT[:])

            # ones block-diag for per-parity partition sum + broadcast
            ones_bd = consts.tile([128, 128], fp32)
            nc.vector.memset(ones_bd[:], 0.0)
            nc.vector.memset(ones_bd[0:64, 0:64], 1.0)
            nc.vector.memset(ones_bd[64:128, 64:128], 1.0)

            # identity64 tiled 16x along free dim (for bs part of RIe)
            iwide = consts.tile([N, TPB, N], f32r)
            for r in range(TPB):
                nc.vector.tensor_copy(iwide[:, r, :], ident[:N, :N])

            # ---- persistent W / RI operand rings ----
            # Outer product + bias as a 2-matmul K-accumulation:
            #   W1 (80, 128): rows 0..16 = lhs of even batches (cols 0..64),
            #     rows 16..80 = bs^T in both column halves;
            #   RIe (80, 1024): rows 0..16 = even-batch rhs block-diagonal
            #     (row t nonzero at cols 64t..64t+64), rows 16..80 =
            #     identity64 tiled 16x;
            #   W2/RIo (16, ...): the same for odd batches.
            # pt = W1^T @ RIe + W2^T @ RIo holds outer(lhs,rhs)+bs for all
            # 32 batches: (parity, i) x (pair, k).
            W1_ring, W2_ring, RIe_ring, RIo_ring = [], [], [], []
            for ri in range(3):
                W1t = consts.tile([80, 128], f32r, name=f"W1{ri}", tag=f"W1{ri}")
                nc.vector.tensor_copy(W1t[:], zeros[0:80, 0:128])
                nc.sync.dma_start(W1t[16:80, 0:64], bsT[:])
                nc.sync.dma_start(W1t[16:80, 64:128], bsT[:])
                W2t = consts.tile([16, 128], f32r, name=f"W2{ri}", tag=f"W2{ri}")
                nc.vector.tensor_copy(W2t[:], zeros[0:16, 0:128])
                RIet = consts.tile([80, 1024], f32r, name=f"RIe{ri}",
                                   tag=f"RIe{ri}")
                nc.vector.tensor_copy(RIet[:], zeros[0:80, :])
                nc.sync.dma_start(
                    RIet[16:80, :], iwide[:].rearrange("p a b -> p (a b)")
                )
                RIot = consts.tile([16, 1024], f32r, name=f"RIo{ri}",
                                   tag=f"RIo{ri}")
                nc.vector.tensor_copy(RIot[:], zeros[0:16, :])
                W1_ring.append(W1t)
                W2_ring.append(W2t)
                RIe_ring.append(RIet)
                RIo_ring.append(RIot)

            def diag_ap(RIt):
                # dims (t16, i64) -> RI[t, t*64 + i]  (diagonal, offset 0)
                return bass.AP(RIt[:].tensor, 0, [[1088, 16], [1, 64]])

            # ---- main loop: software pipeline ----
            st = {}

            def s0_load(g):
                if g in prefetched:
                    st[g] = {"xt": prefetched.pop(g)}
                    return
                xt = xin_p.tile([128, TPB, D], fp32)
                src = x_d[(g % NG) * G2 * N : ((g % NG) + 1) * G2 * N, :].rearrange(
                    "(t p) d -> p t d", p=128
                )
                nc.scalar.dma_start(xt[:], src)
                st[g] = {"xt": xt}

            def s1_xT(g):
                # 32 PE transposes of (128,128) chunks -> PSUM, 8 per 2-bank
                # group; cast back to xTb fp32r (d' on partitions).
                xt = st[g].pop("xt")
                xTb = xT_p.tile([128, TPB, 2, 128], f32r)
                for q in range(4):
                    psT = psT_p.tile([128, 8, 128], fp32, tag="psT")
                    for j in range(8):
                        t = 4 * q + j // 2
                        c = j % 2
                        nc.tensor.matmul(
                            psT[:, j, :],
                            lhsT=xt[:, t, 128 * c : 128 * (c + 1)],
                            rhs=ident[:],
                            is_transpose=True,
                            skip_group_check=True,
                        )
                    dst = xTb[:, 4 * q : 4 * q + 4, :, :].rearrange(
                        "p t c r -> p (t c) r"
                    )
                    nc.vector.tensor_copy(dst, psT[:])
                st[g]["xTb"] = xTb

            def s2_dots(g):
                # lhs/rhs dots for 2048 rows: 4 chains x 2 matmuls (K=256
                # over 2 chunks), N=512 each, fp32r.
                xTb = st[g].pop("xTb")
                psDs = []
                for nh in range(4):
                    psD = psD_p.tile([2, 512], fp32, tag="psD")
                    for c in range(2):
                        nc.tensor.matmul(
                            psD[:],
                            lhsT=w23b[:, c, :],
                            rhs=xTb[:, 4 * nh : 4 * nh + 4, c, :],
                            start=(c == 0),
                            stop=(c == 1),
                        )
                    psDs.append(psD)
                # stage layout: [lr(part), nh4, t4, half, i]
                stage = stg_p.tile([2, 4, 4, 2, 64], f32r)
                for nh in range(4):
                    nc.scalar.copy(
                        stage[:, nh, :, :, :],
                        psDs[nh][:].rearrange("p (t h i) -> p t h i", t=4, h=2),
                    )
                # hop 1: scatter the rhs dot row onto 16 partitions (t-major)
                rstage = rst_p.tile([TPB, 2, 64], f32r)
                nc.sync.dma_start(
                    rstage[:],
                    stage[1:2].rearrange("p n t h i -> p (n t) (h i)"),
                )
                st[g]["stage"] = stage
                st[g]["rstage"] = rstage

            def s2b_wri(g):
                stage = st[g].pop("stage")
                rstage = st[g].pop("rstage")
                ring = g % 3
                nc.sync.dma_start(
                    W1_ring[ring][0:16, 0:64], stage[0:1, :, :, 0, :]
                )
                nc.sync.dma_start(
                    W2_ring[ring][0:16, 64:128], stage[0:1, :, :, 1, :]
                )
                nc.gpsimd.dma_start(diag_ap(RIe_ring[ring]), rstage[:, 0, :])
                nc.gpsimd.dma_start(diag_ap(RIo_ring[ring]), rstage[:, 1, :])
                st[g]["ring"] = ring

            def s3_outer(g):
                ring = st[g].pop("ring")
                pt = big_p.tile([128, 2, 512], fp32, tag="big")
                for u in range(2):
                    nc.tensor.matmul(
                        pt[:, u, :],
                        lhsT=W1_ring[ring][:],
                        rhs=RIe_ring[ring][:, 512 * u : 512 * (u + 1)],
                        start=True,
                        stop=False,
                    )
                    nc.tensor.matmul(
                        pt[:, u, :],
                        lhsT=W2_ring[ring][:],
                        rhs=RIo_ring[ring][:, 512 * u : 512 * (u + 1)],
                        start=False,
                        stop=True,
                    )
                t_sb = tsb_p.tile([128, 1024], f32r)
                nc.scalar.activation(
                    t_sb[:],
                    pt[:].rearrange("p a b -> p (a b)"),
                    mybir.ActivationFunctionType.Tanh,
                )
                st[g]["t_sb"] = t_sb

            def s4_vs(g):
                t_sb = st[g].pop("t_sb")
                ps = big_p.tile([128, 2, 512], fp32, tag="big")
                for u in range(2):
                    nc.tensor.matmul(
                        ps[:, u, :],
                        lhsT=VsBD[:],
                        rhs=t_sb[:, 512 * u : 512 * (u + 1)],
                        start=True,
                        stop=True,
                    )
                e_sb = esb_p.tile([128, TPB, 64], fp32)
                nc.scalar.activation(
                    e_sb[:].rearrange("p a b -> p (a b)"),
                    ps[:].rearrange("p a b -> p (a b)"),
                    mybir.ActivationFunctionType.Exp,
                )
                st[g]["e_sb"] = e_sb

            def s5a_sum(g):
                e_sb = st[g]["e_sb"]
                sums = small_p.tile([128, TPB], fp32, tag="sums")
                nc.vector.tensor_reduce(
                    out=sums[:], in_=e_sb[:], axis=mybir.AxisListType.X, op=add
                )
                st[g]["sums"] = sums

            def s5_soft(g):
                e_sb = st[g].pop("e_sb")
                sums = st[g].pop("sums")
                sb_ps = psD_p.tile([128, TPB], fp32, tag="psD")
                nc.tensor.matmul(
                    sb_ps[:], lhsT=ones_bd[:], rhs=sums[:], start=True, stop=True
                )
                rb = small_p.tile([128, TPB], fp32, tag="rb")
                nc.vector.reciprocal(rb[:], sb_ps[:])
                a2 = asb_p.tile([128, TPB, 64], fp32, tag="a2", name="a2")
                rb_b = rb[:].unsqueeze(-1).broadcast_to((128, TPB, 64))
                nc.gpsimd.tensor_tensor(out=a2[:], in0=e_sb[:], in1=rb_b, op=mult)
                # batch index within a group of 32 is 2*b + h; partition is
                # (h, i)
                dst = out_d[(g % NG) * G2 : (g % NG + 1) * G2].rearrange(
                    "(b two) i k -> two i b k", two=2
                )
                for h in range(2):
                    nc.sync.dma_start(dst[h], a2[64 * h : 64 * h + 64, :, :])
                del st[g]

            lags = [(0, s0_load), (1, s1_xT), (2, s2_dots), (3, s2b_wri),
                    (4, s3_outer), (5, s4_vs), (6, s5a_sum), (7, s5_soft)]
            total = NG * repeat
            for i in range(total + 7):
                for lag, fn in lags:
                    g = i - lag
                    if 0 <= g < total:
                        fn(g)

    nc.compile()
    return nc


def _get_nc():
    if "nc" not in _CACHE:
        _CACHE["nc"] = build_bass()
    return _CACHE["nc"]


def make_in_maps(x, W1, W2, W3, bs, Vs):
    x = np.ascontiguousarray(np.asarray(x, dtype=np.float32))
    w2p = (np.asarray(W2, np.float32)[:, 0] * np.asarray(W1, np.float32)[0]).reshape(1, D)
    w3 = np.asarray(W3, np.float32).reshape(1, D)
    bsm = np.ascontiguousarray(np.asarray(bs, np.float32).reshape(N, N))
    vs = np.ascontiguousarray(np.asarray(Vs, np.float32))
    in_maps = []
    for c in range(NCORES):
        shard = x[c * BS : (c + 1) * BS].reshape(BS * N, D)
        in_maps.append(
            {
                "x": np.ascontiguousarray(shard),
                "w2p": w2p,
                "w3": w3,
                "bsm": bsm,
                "vs": vs,
            }
        )
    return in_maps


def kernel(x, W1, W2, W3, bs, Vs):
    from concourse.bass_utils import run_bass_kernel_spmd

    nc = _get_nc()
    in_maps = make_in_maps(x, W1, W2, W3, bs, Vs)
    res = run_bass_kernel_spmd(nc, in_maps, core_ids=list(range(NCORES)))
    out = np.concatenate([r["out"] for r in res.results], axis=0)
    return out.astype(np.float32)


if __name__ == "__main__":
    nc = build_bass()
    print("built ok:", nc)
